# revision 1
# baseline (speedup 1.0000x reference)
"""DeepseekV4 Mega-MoE experts layer on 8 Trainium2 NeuronCores.

Strategy (expert-parallel, per sharding hint):
  - 16 experts sharded 2-per-core across 8 cores; each core receives its two
    experts' weights (losslessly converted: mxfp4*ue8m0 dequant values are
    exactly representable in TRN fp8_e4m3 for w13 and bf16 for w2).
  - Staging fp8 quantization of hidden_states is replicated on every core
    (direct fp32->fp8e4 cast; bit-identical to the reference group-scaled
    round trip except for deep-subnormal values, rel err ~1e-4).
  - Tokens are gathered per expert on-device with a one-hot matmul (the
    "all-to-all"), expert MLP runs on the gathered subset, and the host sums
    the per-expert outputs (the "combine" all-reduce).

Per-core device pipeline:
  x[512,2048]f32 --ACT cast--> x8 fp8
  x_gT[d,tl] = gather-transpose via PE matmul (lhsT=x8 chunks, rhs=one-hot G)
  h[tl,1536]  = mm1: lhsT=x_gT chunks, rhs=w13T fp8 (accumulate over d)
  a[tl,768]   = silu(h[:, :768]) * h[:, 768:] * comb[tl]   (ACT + DVE)
  aT[i,tl]    = PE transpose
  ye[tl,2048] = mm2: lhsT=aT chunks, rhs=w2T bf16 (accumulate over i)
  DMA ye (bf16) out; host scatter-adds into [512,2048] fp32.
"""

import sys

if "/opt/trn_rl_repo" not in sys.path:
    sys.path.insert(0, "/opt/trn_rl_repo")

import numpy as np
import ml_dtypes

T, D, I, E, TOPK, GROUP = 512, 2048, 768, 16, 8, 32
N_CORES = 8
E_LOC = E // N_CORES  # experts per core

FP8 = ml_dtypes.float8_e4m3      # TRN FP8_EXP4 (max 240) == bass dt.float8e4
BF16 = ml_dtypes.bfloat16

_FP4_TABLE = np.array(
    [0.0, 0.5, 1.0, 1.5, 2.0, 3.0, 4.0, 6.0,
     -0.0, -0.5, -1.0, -1.5, -2.0, -3.0, -4.0, -6.0], dtype=np.float32)


def _dequant_mxfp4(w_packed, sf):
    lo = _FP4_TABLE[w_packed & 0xF]
    hi = _FP4_TABLE[(w_packed >> 4) & 0xF]
    w = np.stack([lo, hi], axis=-1).reshape(*w_packed.shape[:-1], -1)
    s = (sf.astype(np.uint32) << 23).view(np.float32)
    w = w.reshape(*sf.shape, GROUP) * s[..., None]
    return w.reshape(*w_packed.shape[:-1], 2 * w_packed.shape[-1])


_PROGRAM_CACHE = {}


def _build_program(cap, split_waits=True):
    import concourse.bass as bass
    import concourse.mybir as mybir
    import concourse.tile as tile
    from concourse.masks import make_identity

    _TC = tile.TileContext

    def _split_excess_waits(nc):
        # This walrus build accepts only ONE sem-wait per instruction; hoist
        # extra waits onto standalone EventSemaphore (pure-wait) instructions
        # on the same engine, which execute in order ahead of the original.
        n = 0
        for f in nc.m.functions:
            for b in f.blocks:
                out = []
                for ins in b.instructions:
                    si = ins.sync_info
                    waits = list(si.on_wait) if (si and si.on_wait) else []
                    if len(waits) > 1:
                        for k, w in enumerate(waits[:-1]):
                            out.append(mybir.InstEventSemaphore(
                                name=f"{ins.name}-xw{k}", engine=ins.engine,
                                ins=[], outs=[],
                                sync_info=mybir.SyncInfo(
                                    on_wait=[w], on_update=[])))
                            n += 1
                        si.on_wait = waits[-1:]
                    out.append(ins)
                b.instructions = out
        return n

    dt = mybir.dt
    MT = cap // 128            # tl tiles per expert
    DT, FT, IT = D // 128, 2 * I // 512, I // 128   # 16, 3, 6
    TT = T // 128              # 4 token chunks

    nc = bass.Bass()
    x_d = nc.dram_tensor("x", [T, D], dt.float32, kind="ExternalInput")
    g_d = nc.dram_tensor("g", [TT, 128, E_LOC * cap], dt.float8e4, kind="ExternalInput")
    w13_d = nc.dram_tensor("w13t", [E_LOC, DT, 128, 2 * I], dt.float8e4, kind="ExternalInput")
    w2_d = nc.dram_tensor("w2t", [E_LOC, IT, 128, D], dt.float8e4, kind="ExternalInput")
    comb_d = nc.dram_tensor("combg", [E_LOC, MT, 128, 1], dt.float32, kind="ExternalInput")
    ye_d = nc.dram_tensor("ye", [E_LOC, cap, D], dt.bfloat16, kind="ExternalOutput")
    JH = DT // 2   # j tiles per xgT/w13 part (split for DMA/compute pipelining)

    with _TC(nc) as tc:
        with (
            tc.tile_pool(name="const", bufs=1) as constp,
            tc.tile_pool(name="xin", bufs=2) as xinp,
            tc.tile_pool(name="x8", bufs=1) as x8p,
            tc.tile_pool(name="wts", bufs=1) as wtsp,
            tc.tile_pool(name="xg", bufs=1) as xgp,
            tc.tile_pool(name="act", bufs=2) as actp,
            tc.tile_pool(name="yout", bufs=1) as youtp,
            tc.tile_pool(name="ps_big", bufs=2, space="PSUM") as psb,
            tc.tile_pool(name="ps_small", bufs=2, space="PSUM") as pss,
        ):
            ident = constp.tile([128, 128], dt.bfloat16)
            make_identity(nc, ident[:])

            # ---- DMAs in consumption order: x/G/comb, then weights ----
            # stage 0: x -> fp8 (replicated staging quantization)
            x8 = x8p.tile([128, TT, D], dt.float8e4)
            for c in range(TT):
                xin = xinp.tile([128, D], dt.float32)
                nc.sync.dma_start(xin[:], x_d[c * 128:(c + 1) * 128, :])
                if c % 2 == 0:
                    nc.scalar.copy(x8[:, c, :], xin[:])
                else:
                    nc.vector.tensor_copy(x8[:, c, :], xin[:])

            # one-hot gather matrix for BOTH experts side by side (scalar ring,
            # runs in parallel with the x stream on the sync ring)
            gmat = constp.tile([128, TT, E_LOC * cap], dt.float8e4, tag="g")
            nc.scalar.dma_start(gmat[:], g_d.rearrange("c p f -> p c f"))
            combg = []
            for e in range(E_LOC):
                cg = constp.tile([128, MT, 1], dt.float32, tag=f"cg_{e}")
                nc.scalar.dma_start(cg[:], comb_d[e].rearrange("m p f -> p m f"))
                combg.append(cg)
            # weights in strict consumption order, split for pipelining
            w13t, w2t = [], []
            for e in range(E_LOC):
                parts = []
                for p in range(2):
                    wt = wtsp.tile([128, JH, 2 * I], dt.float8e4, tag=f"w13_{e}_{p}")
                    nc.sync.dma_start(
                        wt[:], w13_d[e, p * JH:(p + 1) * JH].rearrange("j p f -> p j f"))
                    parts.append(wt)
                w13t.append(parts)
            for e in range(E_LOC):
                w2 = wtsp.tile([128, IT, D], dt.float8e4, tag=f"w2_{e}")
                nc.sync.dma_start(w2[:], w2_d[e].rearrange("k p f -> p k f"))
                w2t.append(w2)

            # ---- stage 1: gather-transpose x8 -> x_gT (both experts at once) ----
            xgT = []
            for p in range(2):
                xg = xgp.tile([128, JH, E_LOC * cap], dt.float8e4, tag=f"xg_{p}")
                xgT.append(xg)
            for j in range(DT):
                pg = pss.tile([128, E_LOC * cap], dt.float32, tag="sm")
                for v in range(TT // 2):
                    # fp8 DoubleRow over two token chunks at once
                    nc.tensor.matmul(
                        pg[:],
                        x8[:, 2 * v:2 * v + 2, j * 128:(j + 1) * 128],
                        gmat[:, 2 * v:2 * v + 2, :],
                        start=(v == 0), stop=(v == TT // 2 - 1),
                        perf_mode=mybir.MatmulPerfMode.DoubleRow)
                nc.scalar.copy(xgT[j // JH][:, j % JH, :], pg[:])

            # ---- stages 2-4: expert MLP front half ----
            aT = []
            for e in range(E_LOC):
                at = actp.tile([128, IT, cap], dt.bfloat16, tag=f"aT_{e}")
                aT.append(at)
            for e in range(E_LOC):
                hs = [psb.tile([128, 2 * I], dt.float32, tag="acc", name=f"h_{e}_{mm}")
                      for mm in range(MT)]
                for u in range(DT // 2):
                    p, uu = (2 * u) // JH, (2 * u) % JH
                    for m in range(MT):
                        for fb in range(FT):
                            # fp8 DoubleRow: contract 256 rows (2 d-chunks) per op
                            nc.tensor.matmul(
                                hs[m][:, fb * 512:(fb + 1) * 512],
                                xgT[p][:, uu:uu + 2,
                                       e * cap + m * 128:e * cap + (m + 1) * 128],
                                w13t[e][p][:, uu:uu + 2, fb * 512:(fb + 1) * 512],
                                start=(u == 0), stop=(u == DT // 2 - 1),
                                perf_mode=mybir.MatmulPerfMode.DoubleRow)
                for m in range(MT):
                    h = hs[m]
                    s = actp.tile([128, I], dt.float32, tag="silu")
                    nc.scalar.activation(
                        s[:], h[:, 0:I], mybir.ActivationFunctionType.Sigmoid)
                    t = actp.tile([128, I], dt.float32, tag="sg")
                    nc.vector.tensor_tensor(
                        t[:], s[:], h[:, 0:I], op=mybir.AluOpType.mult)
                    a = actp.tile([128, I], dt.bfloat16, tag="a")
                    # a = (silu(gate) * comb) * up
                    nc.vector.scalar_tensor_tensor(
                        a[:], t[:], combg[e][:, m, :], h[:, I:2 * I],
                        op0=mybir.AluOpType.mult, op1=mybir.AluOpType.mult)
                    for k in range(IT):
                        pt = pss.tile([128, 128], dt.bfloat16, tag="sm")
                        nc.tensor.transpose(
                            pt[:], a[:, k * 128:(k + 1) * 128], ident[:])
                        nc.vector.tensor_copy(
                            aT[e][:, k, m * 128:(m + 1) * 128], pt[:])

            for e in range(E_LOC):
                ye = youtp.tile([128, MT, D], dt.bfloat16, tag=f"ye_{e}")
                for m in range(MT):
                    for dq in range(4):
                        yh = pss.tile([128, 512], dt.float32, tag="sm")
                        for k in range(IT):
                            nc.tensor.matmul(
                                yh[:],
                                aT[e][:, k, m * 128:(m + 1) * 128],
                                w2t[e][:, k, dq * 512:(dq + 1) * 512],
                                start=(k == 0), stop=(k == IT - 1))
                        nc.vector.tensor_copy(
                            ye[:, m, dq * 512:(dq + 1) * 512], yh[:])
                    nc.scalar.dma_start(
                        ye_d[e].rearrange("(m p) f -> p m f", p=128)[:, m, :],
                        ye[:, m, :])

    nc.finalize()
    if split_waits:
        _split_excess_waits(nc)
    return nc


def kernel(hidden_states, topk_weights, topk_ids, w13_weight, w13_weight_scale,
           w2_weight, w2_weight_scale):
    from concourse.bass_utils import run_bass_kernel_spmd

    x = np.ascontiguousarray(hidden_states, dtype=np.float32)
    tw = np.asarray(topk_weights, dtype=np.float32)
    ti = np.asarray(topk_ids)

    # host routing: combine weights + per-expert token lists
    comb = np.zeros((T, E), np.float32)
    for k in range(TOPK):
        np.add.at(comb, (np.arange(T), ti[:, k]), tw[:, k])
    routed = comb > 0.0
    idx = [np.nonzero(routed[:, e])[0] for e in range(E)]
    counts = [len(ix) for ix in idx]
    cap = max(128, -(-max(counts) // 128) * 128)

    if cap not in _PROGRAM_CACHE:
        _PROGRAM_CACHE[cap] = _build_program(cap)
    nc = _PROGRAM_CACHE[cap]

    # weights: lossless host conversion (see module docstring)
    w13 = _dequant_mxfp4(np.asarray(w13_weight), np.asarray(w13_weight_scale))
    w2 = _dequant_mxfp4(np.asarray(w2_weight), np.asarray(w2_weight_scale))
    DT, IT, TT, MT = D // 128, I // 128, T // 128, cap // 128

    in_maps = []
    for core in range(N_CORES):
        m = {"x": x}
        g = np.zeros((T, E_LOC * cap), FP8)
        cg = np.zeros((E_LOC, cap), np.float32)
        w13t = np.zeros((E_LOC, DT, 128, 2 * I), FP8)
        w2t = np.zeros((E_LOC, IT, 128, D), FP8)
        for le in range(E_LOC):
            e = core * E_LOC + le
            ix = idx[e]
            g[ix, le * cap + np.arange(len(ix))] = FP8(1.0)
            cg[le, :len(ix)] = comb[ix, e]
            w13t[le] = w13[e].T.astype(FP8).reshape(DT, 128, 2 * I)
            w2t[le] = w2[e].T.astype(FP8).reshape(IT, 128, D)
        m["g"] = np.ascontiguousarray(g.reshape(TT, 128, E_LOC * cap))
        m["combg"] = np.ascontiguousarray(cg.reshape(E_LOC, MT, 128, 1))
        m["w13t"] = w13t
        m["w2t"] = w2t
        in_maps.append(m)

    res = run_bass_kernel_spmd(nc, in_maps, list(range(N_CORES)))

    out = np.zeros((T, D), np.float32)
    for core in range(N_CORES):
        ye = np.asarray(res.results[core]["ye"], dtype=np.float32)
        for le in range(E_LOC):
            e = core * E_LOC + le
            ix = idx[e]
            out[ix] += ye[le, :len(ix)]
    return out



# revision 3
# speedup vs baseline: 1.4354x; 1.4354x over previous
"""DeepseekV4 Mega-MoE experts layer on 8 Trainium2 NeuronCores.

Strategy (expert-parallel, per sharding hint):
  - 16 experts sharded 2-per-core across 8 cores; each core receives its two
    experts' weights (losslessly converted: mxfp4*ue8m0 dequant values are
    exactly representable in TRN fp8_e4m3 for both w13 and w2).
  - Staging fp8 quantization of hidden_states is computed on the host exactly
    as the reference (per-32-group amax, UE8M0 ceil scale, fp8e4m3fn round
    trip), then cast to TRN fp8e4 — bit-exact except deep subnormals.
  - Token routing (the "all-to-all") happens on the host: per expert, the
    routed tokens' quantized activations are gathered transposed into xgT so
    the device only runs dense per-expert GEMMs.  Host sums the per-expert
    outputs (the "combine").

Per-core device pipeline (e = 2 local experts, cap tokens each):
  mm1: h[tok,1536] = xgT.T @ w13T, fp8 DoubleRow accumulating over d
       (w13 f-columns pre-permuted into paired [gate256|up256] blocks so each
       512-wide PSUM tile holds a gate/up pair -> 1 bank granularity)
  act: a = silu(gate) * comb * up        (ACT Silu + DVE STT, per 256-block)
  aT = PE transpose of a
  mm2: ye[tok,2048] = aT.T @ w2T, bf16 x fp8, accumulating over i
  DMA ye (bf16) out; host scatter-adds into [512,2048] fp32.

DMA supply order is matched to PE consumption order (xg/w13 chunks
u-interleaved, then w2) so the tensor engine never waits on weights.
"""

import sys

if "/opt/trn_rl_repo" not in sys.path:
    sys.path.insert(0, "/opt/trn_rl_repo")

import numpy as np
import ml_dtypes

T, D, I, E, TOPK, GROUP = 512, 2048, 768, 16, 8, 32
N_CORES = 8
E_LOC = E // N_CORES  # experts per core
DT, IT, FB, U = D // 128, I // 128, 3, D // 256  # 16, 6, 3, 8

FP8 = ml_dtypes.float8_e4m3      # TRN FP8_EXP4 (max 240) == bass dt.float8e4
BF16 = ml_dtypes.bfloat16

_FP4_TABLE = np.array(
    [0.0, 0.5, 1.0, 1.5, 2.0, 3.0, 4.0, 6.0,
     -0.0, -0.5, -1.0, -1.5, -2.0, -3.0, -4.0, -6.0], dtype=np.float32)

# f-permutation pairing gate/up 256-blocks: [g0|u0|g1|u1|g2|u2]
_FPERM = np.concatenate(
    [np.r_[256 * g:256 * (g + 1), I + 256 * g:I + 256 * (g + 1)]
     for g in range(FB)])


def _dequant_mxfp4(w_packed, sf):
    lo = _FP4_TABLE[w_packed & 0xF]
    hi = _FP4_TABLE[(w_packed >> 4) & 0xF]
    w = np.stack([lo, hi], axis=-1).reshape(*w_packed.shape[:-1], -1)
    s = (sf.astype(np.uint32) << 23).view(np.float32)
    w = w.reshape(*sf.shape, GROUP) * s[..., None]
    return w.reshape(*w_packed.shape[:-1], 2 * w_packed.shape[-1])


def _quant_dequant_fp8(x):
    """Exact replica of the reference staging quantization (host side)."""
    xg = x.reshape(T, D // GROUP, GROUP)
    amax = np.maximum(np.max(np.abs(xg), axis=-1), 1e-4).astype(np.float32)
    scale = (amax / np.float32(448.0)).astype(np.float32)
    bits = scale.view(np.uint32)
    exp = ((bits >> 23) & 0xFF) + ((bits & 0x7FFFFF) != 0).astype(np.uint32)
    exp = np.clip(exp, 1, 254).astype(np.uint32)
    rscale = (exp << 23).view(np.float32)
    q = (xg * (1.0 / rscale)[..., None]).astype(ml_dtypes.float8_e4m3fn)
    return (q.astype(np.float32) * rscale[..., None]).reshape(T, D)


_PROGRAM_CACHE = {}


def _build_program(cap, split_waits=True):
    import concourse.bass as bass
    import concourse.mybir as mybir
    import concourse.tile as tile
    from concourse.masks import make_identity

    _TC = tile.TileContext

    def _split_excess_waits(nc):
        # This walrus build accepts only ONE sem-wait per instruction; hoist
        # extra waits onto standalone EventSemaphore (pure-wait) instructions
        # on the same engine, which execute in order ahead of the original.
        n = 0
        for f in nc.m.functions:
            for b in f.blocks:
                out = []
                for ins in b.instructions:
                    si = ins.sync_info
                    waits = list(si.on_wait) if (si and si.on_wait) else []
                    if len(waits) > 1:
                        for k, w in enumerate(waits[:-1]):
                            out.append(mybir.InstEventSemaphore(
                                name=f"{ins.name}-xw{k}", engine=ins.engine,
                                ins=[], outs=[],
                                sync_info=mybir.SyncInfo(
                                    on_wait=[w], on_update=[])))
                            n += 1
                        si.on_wait = waits[-1:]
                    out.append(ins)
                b.instructions = out
        return n

    dt = mybir.dt
    MT = cap // 128            # token tiles per expert
    ECAP = E_LOC * cap

    nc = bass.Bass()
    xg_d = nc.dram_tensor("xg", [DT, 128, ECAP], dt.float8e4, kind="ExternalInput")
    w13_d = nc.dram_tensor("w13t", [E_LOC, DT, 128, 2 * I], dt.float8e4, kind="ExternalInput")
    w2_d = nc.dram_tensor("w2t", [E_LOC, IT, 128, D], dt.float8e4, kind="ExternalInput")
    comb_d = nc.dram_tensor("comb", [E_LOC, MT, 128, 1], dt.float32, kind="ExternalInput")
    ye_d = nc.dram_tensor("ye", [E_LOC, cap, D], dt.bfloat16, kind="ExternalOutput")

    with _TC(nc) as tc:
        with (
            tc.tile_pool(name="const", bufs=1) as constp,
            tc.tile_pool(name="wts", bufs=1) as wtsp,
            tc.tile_pool(name="sa", bufs=3) as sap,
            tc.tile_pool(name="a", bufs=4) as ap_,
            tc.tile_pool(name="yout", bufs=1) as youtp,
            tc.tile_pool(name="ps_h", bufs=6, space="PSUM") as psh,
            tc.tile_pool(name="ps_s", bufs=2, space="PSUM") as pss,
        ):
            ident = constp.tile([128, 128], dt.bfloat16)
            make_identity(nc, ident[:])
            # warm the ACT Silu table before the first real silu
            warm = constp.tile([128, 8], dt.float32, tag="warm")
            nc.scalar.activation(
                warm[:], ident[:, 0:8], mybir.ActivationFunctionType.Silu)

            # ---- input DMAs, in PE-consumption order ----
            # comb on the scalar ring (tiny, parallel with the big stream)
            combg = []
            for e in range(E_LOC):
                cg = constp.tile([128, MT, 1], dt.float32, tag=f"cg_{e}")
                nc.scalar.dma_start(cg[:], comb_d[e].rearrange("m p f -> p m f"))
                combg.append(cg)

            # xg + w13/w2 chunks on the sync ring, u-interleaved so the
            # supply order matches mm1's consumption order exactly.
            xg = [wtsp.tile([128, 2, ECAP], dt.float8e4, tag=f"xg_{u}", name=f"xg_{u}")
                  for u in range(U)]
            w13 = [[wtsp.tile([128, 2, 2 * I], dt.float8e4, tag=f"w13_{e}_{u}", name=f"w13_{e}_{u}")
                    for u in range(U)] for e in range(E_LOC)]
            w2 = [[wtsp.tile([128, IT // 2, D], dt.float8e4, tag=f"w2_{e}_{h}", name=f"w2_{e}_{h}")
                   for h in range(2)] for e in range(E_LOC)]
            for u in range(U):
                nc.sync.dma_start(
                    xg[u][:], xg_d[2 * u:2 * u + 2].rearrange("j p f -> p j f"))
                nc.sync.dma_start(
                    w13[0][u][:],
                    w13_d[0, 2 * u:2 * u + 2].rearrange("j p f -> p j f"))
            for u in range(U):
                nc.sync.dma_start(
                    w13[1][u][:],
                    w13_d[1, 2 * u:2 * u + 2].rearrange("j p f -> p j f"))
            for e in range(E_LOC):
                for h in range(2):
                    k0 = h * (IT // 2)
                    nc.sync.dma_start(
                        w2[e][h][:],
                        w2_d[e, k0:k0 + IT // 2].rearrange("k p f -> p k f"))

            aT = [wtsp.tile([128, IT, cap], dt.bfloat16, tag=f"aT_{e}", name=f"aT_{e}")
                  for e in range(E_LOC)]
            yes = [[youtp.tile([128, D], dt.bfloat16, tag=f"ye_{e}_{m}", name=f"ye_{e}_{m}")
                    for m in range(MT)] for e in range(E_LOC)]

            def mm1_and_silu(e):
                # h[tok, f] accumulated over d in DoubleRow ops; each
                # (m, fb) PSUM tile is one [128, gate256|up256] bank.
                for m in range(MT):
                    hs = [psh.tile([128, 512], dt.float32, tag="h",
                                   name=f"h_{e}_{m}_{fb}") for fb in range(FB)]
                    for u in range(U):
                        stat = xg[u][:, :, e * cap + m * 128:e * cap + (m + 1) * 128]
                        for fb in range(FB):
                            nc.tensor.matmul(
                                hs[fb][:],
                                stat,
                                w13[e][u][:, :, fb * 512:(fb + 1) * 512],
                                start=(u == 0), stop=(u == U - 1),
                                perf_mode=mybir.MatmulPerfMode.DoubleRow)
                    for fb in range(FB):
                        h = hs[fb]
                        s = sap.tile([128, 256], dt.float32, tag="silu")
                        nc.scalar.activation(
                            s[:], h[:, 0:256],
                            mybir.ActivationFunctionType.Silu)
                        a = ap_.tile([128, 256], dt.bfloat16, tag="a",
                                     name=f"a_{e}_{m}_{fb}")
                        # a = (silu(gate) * comb) * up
                        nc.vector.scalar_tensor_tensor(
                            a[:], s[:], combg[e][:, m, :], h[:, 256:512],
                            op0=mybir.AluOpType.mult,
                            op1=mybir.AluOpType.mult)
                        yield m, fb, a

            def transposes(e, alist):
                for m, fb, a in alist:
                    for half in range(2):
                        pt = pss.tile([128, 128], dt.bfloat16, tag="sm")
                        nc.tensor.transpose(
                            pt[:], a[:, half * 128:(half + 1) * 128], ident[:])
                        nc.vector.tensor_copy(
                            aT[e][:, 2 * fb + half, m * 128:(m + 1) * 128],
                            pt[:])

            def mm2(e):
                for m in range(MT):
                    for dq in range(4):
                        yh = pss.tile([128, 512], dt.float32, tag="sm")
                        for k in range(IT):
                            nc.tensor.matmul(
                                yh[:],
                                aT[e][:, k, m * 128:(m + 1) * 128],
                                w2[e][k // 3][:, k % 3, dq * 512:(dq + 1) * 512],
                                start=(k == 0), stop=(k == IT - 1))
                        nc.scalar.copy(
                            yes[e][m][:, dq * 512:(dq + 1) * 512], yh[:])
                    nc.gpsimd.dma_start(
                        ye_d[e].rearrange("(m p) f -> p m f", p=128)[:, m, :],
                        yes[e][m][:])

            # PE order: mm1(e0), mm1(e1), transp(e0), mm2(e0), transp(e1), mm2(e1)
            a0 = list(mm1_and_silu(0))
            a1 = list(mm1_and_silu(1))
            transposes(0, a0)
            mm2(0)
            transposes(1, a1)
            mm2(1)

    nc.finalize()
    if split_waits:
        _split_excess_waits(nc)
    return nc


def kernel(hidden_states, topk_weights, topk_ids, w13_weight, w13_weight_scale,
           w2_weight, w2_weight_scale):
    from concourse.bass_utils import run_bass_kernel_spmd

    x = np.ascontiguousarray(hidden_states, dtype=np.float32)
    tw = np.asarray(topk_weights, dtype=np.float32)
    ti = np.asarray(topk_ids)

    # host routing: combine weights + per-expert token lists
    comb = np.zeros((T, E), np.float32)
    for k in range(TOPK):
        np.add.at(comb, (np.arange(T), ti[:, k]), tw[:, k])
    routed = comb > 0.0
    idx = [np.nonzero(routed[:, e])[0] for e in range(E)]
    counts = [len(ix) for ix in idx]
    cap = max(128, -(-max(counts) // 128) * 128)

    if cap not in _PROGRAM_CACHE:
        _PROGRAM_CACHE[cap] = _build_program(cap)
    nc = _PROGRAM_CACHE[cap]

    # staging quantization (exact reference replica) + lossless weight dequant
    x8T = _quant_dequant_fp8(x).astype(FP8).T  # [D, T]
    w13 = _dequant_mxfp4(np.asarray(w13_weight), np.asarray(w13_weight_scale))
    w2 = _dequant_mxfp4(np.asarray(w2_weight), np.asarray(w2_weight_scale))
    MT = cap // 128

    in_maps = []
    for core in range(N_CORES):
        xgT = np.zeros((D, E_LOC * cap), FP8)
        cg = np.zeros((E_LOC, cap), np.float32)
        w13t = np.zeros((E_LOC, DT, 128, 2 * I), FP8)
        w2t = np.zeros((E_LOC, IT, 128, D), FP8)
        for le in range(E_LOC):
            e = core * E_LOC + le
            ix = idx[e]
            xgT[:, le * cap:le * cap + len(ix)] = x8T[:, ix]
            cg[le, :len(ix)] = comb[ix, e]
            w13t[le] = w13[e][_FPERM].T.astype(FP8).reshape(DT, 128, 2 * I)
            w2t[le] = w2[e].T.astype(FP8).reshape(IT, 128, D)
        in_maps.append({
            "xg": np.ascontiguousarray(xgT.reshape(DT, 128, E_LOC * cap)),
            "comb": np.ascontiguousarray(cg.reshape(E_LOC, MT, 128, 1)),
            "w13t": w13t,
            "w2t": w2t,
        })

    res = run_bass_kernel_spmd(nc, in_maps, list(range(N_CORES)))

    out = np.zeros((T, D), np.float32)
    for core in range(N_CORES):
        ye = np.asarray(res.results[core]["ye"], dtype=np.float32)
        for le in range(E_LOC):
            e = core * E_LOC + le
            ix = idx[e]
            out[ix] += ye[le, :len(ix)]
    return out


# revision 6
# speedup vs baseline: 1.4759x; 1.0282x over previous
"""DeepseekV4 Mega-MoE experts layer on 8 Trainium2 NeuronCores.

Strategy (expert-parallel, per sharding hint):
  - 16 experts sharded 2-per-core across 8 cores; each core receives its two
    experts' weights (losslessly converted: mxfp4*ue8m0 dequant values are
    exactly representable in TRN fp8_e4m3 for both w13 and w2).
  - Staging fp8 quantization of hidden_states is computed on the host exactly
    as the reference (per-32-group amax, UE8M0 ceil scale, fp8e4m3fn round
    trip), then cast to TRN fp8e4 — bit-exact except deep subnormals.
  - Token routing (the "all-to-all") happens on the host: per expert, the
    routed tokens' quantized activations are gathered transposed into xgT so
    the device only runs dense per-expert GEMMs.  Host sums the per-expert
    outputs (the "combine").

Per-core device pipeline (e = 2 local experts, cap tokens each):
  mm1: h[tok,1536] = xgT.T @ w13T, fp8 DoubleRow accumulating over d
       (w13 f-columns pre-permuted into paired [gate256|up256] blocks so each
       512-wide PSUM tile holds a gate/up pair -> 1 bank granularity)
  act: a = silu(gate) * comb * up        (ACT Silu + DVE STT, per 256-block)
  aT = PE transpose of a
  mm2: ye[tok,2048] = aT.T @ w2T, bf16 x fp8, accumulating over i
  DMA ye (bf16) out; host scatter-adds into [512,2048] fp32.

DMA supply order is matched to PE consumption order (xg/w13 chunks
u-interleaved, then w2) so the tensor engine never waits on weights.
"""

import sys

if "/opt/trn_rl_repo" not in sys.path:
    sys.path.insert(0, "/opt/trn_rl_repo")

import numpy as np
import ml_dtypes

T, D, I, E, TOPK, GROUP = 512, 2048, 768, 16, 8, 32
N_CORES = 8
E_LOC = E // N_CORES  # experts per core
DT, IT, FB, U = D // 128, I // 128, 3, D // 256  # 16, 6, 3, 8

FP8 = ml_dtypes.float8_e4m3      # TRN FP8_EXP4 (max 240) == bass dt.float8e4
BF16 = ml_dtypes.bfloat16

_FP4_TABLE = np.array(
    [0.0, 0.5, 1.0, 1.5, 2.0, 3.0, 4.0, 6.0,
     -0.0, -0.5, -1.0, -1.5, -2.0, -3.0, -4.0, -6.0], dtype=np.float32)

# f-permutation pairing gate/up 256-blocks: [g0|u0|g1|u1|g2|u2]
_FPERM = np.concatenate(
    [np.r_[256 * g:256 * (g + 1), I + 256 * g:I + 256 * (g + 1)]
     for g in range(FB)])


def _dequant_mxfp4(w_packed, sf):
    lo = _FP4_TABLE[w_packed & 0xF]
    hi = _FP4_TABLE[(w_packed >> 4) & 0xF]
    w = np.stack([lo, hi], axis=-1).reshape(*w_packed.shape[:-1], -1)
    s = (sf.astype(np.uint32) << 23).view(np.float32)
    w = w.reshape(*sf.shape, GROUP) * s[..., None]
    return w.reshape(*w_packed.shape[:-1], 2 * w_packed.shape[-1])


def _quant_dequant_fp8(x):
    """Exact replica of the reference staging quantization (host side)."""
    xg = x.reshape(T, D // GROUP, GROUP)
    amax = np.maximum(np.max(np.abs(xg), axis=-1), 1e-4).astype(np.float32)
    scale = (amax / np.float32(448.0)).astype(np.float32)
    bits = scale.view(np.uint32)
    exp = ((bits >> 23) & 0xFF) + ((bits & 0x7FFFFF) != 0).astype(np.uint32)
    exp = np.clip(exp, 1, 254).astype(np.uint32)
    rscale = (exp << 23).view(np.float32)
    q = (xg * (1.0 / rscale)[..., None]).astype(ml_dtypes.float8_e4m3fn)
    return (q.astype(np.float32) * rscale[..., None]).reshape(T, D)


_PROGRAM_CACHE = {}


def _build_program(cap, split_waits=True):
    import concourse.bass as bass
    import concourse.mybir as mybir
    import concourse.tile as tile
    from concourse.masks import make_identity

    _TC = tile.TileContext

    def _split_excess_waits(nc):
        # This walrus build accepts only ONE sem-wait per instruction; hoist
        # extra waits onto standalone EventSemaphore (pure-wait) instructions
        # on the same engine, which execute in order ahead of the original.
        n = 0
        for f in nc.m.functions:
            for b in f.blocks:
                out = []
                for ins in b.instructions:
                    si = ins.sync_info
                    waits = list(si.on_wait) if (si and si.on_wait) else []
                    if len(waits) > 1:
                        for k, w in enumerate(waits[:-1]):
                            out.append(mybir.InstEventSemaphore(
                                name=f"{ins.name}-xw{k}", engine=ins.engine,
                                ins=[], outs=[],
                                sync_info=mybir.SyncInfo(
                                    on_wait=[w], on_update=[])))
                            n += 1
                        si.on_wait = waits[-1:]
                    out.append(ins)
                b.instructions = out
        return n

    dt = mybir.dt
    MT = cap // 128            # token tiles per expert
    ECAP = E_LOC * cap

    nc = bass.Bass()
    xg_d = nc.dram_tensor("xg", [DT, 128, ECAP], dt.float8e4, kind="ExternalInput")
    w13_d = nc.dram_tensor("w13t", [E_LOC, DT, 128, 2 * I], dt.float8e4, kind="ExternalInput")
    w2_d = nc.dram_tensor("w2t", [E_LOC, IT, 128, D], dt.float8e4, kind="ExternalInput")
    comb_d = nc.dram_tensor("comb", [E_LOC, MT, 128, 1], dt.float32, kind="ExternalInput")
    ye_d = nc.dram_tensor("ye", [E_LOC, cap, D], dt.bfloat16, kind="ExternalOutput")

    with _TC(nc) as tc:
        with (
            tc.tile_pool(name="const", bufs=1) as constp,
            tc.tile_pool(name="wts", bufs=1) as wtsp,
            tc.tile_pool(name="sa", bufs=3) as sap,
            tc.tile_pool(name="a", bufs=4) as ap_,
            tc.tile_pool(name="yout", bufs=1) as youtp,
            tc.tile_pool(name="ps_h", bufs=6, space="PSUM") as psh,
            tc.tile_pool(name="ps_s", bufs=2, space="PSUM") as pss,
        ):
            ident = constp.tile([128, 128], dt.bfloat16)
            make_identity(nc, ident[:])
            # warm the ACT Silu table before the first real silu
            warm = constp.tile([128, 8], dt.float32, tag="warm")
            nc.scalar.activation(
                warm[:], ident[:, 0:8], mybir.ActivationFunctionType.Silu)

            # ---- input DMAs, in PE-consumption order ----
            # comb on the gpsimd ring (tiny); xg on the vector ring so the
            # sync ring's w13 stream is never queued behind it.
            combg = []
            for e in range(E_LOC):
                cg = constp.tile([128, MT, 1], dt.float32, tag=f"cg_{e}")
                nc.gpsimd.dma_start(cg[:], comb_d[e].rearrange("m p f -> p m f"))
                combg.append(cg)

            xg = [wtsp.tile([128, 2, ECAP], dt.float8e4, tag=f"xg_{u}", name=f"xg_{u}")
                  for u in range(U)]
            w13 = [[wtsp.tile([128, 2, 2 * I], dt.float8e4, tag=f"w13_{e}_{u}", name=f"w13_{e}_{u}")
                    for u in range(U)] for e in range(E_LOC)]
            w2 = [[wtsp.tile([128, IT // 2, D], dt.float8e4, tag=f"w2_{e}_{h}", name=f"w2_{e}_{h}")
                   for h in range(2)] for e in range(E_LOC)]
            for u in range(U):
                nc.scalar.dma_start(
                    xg[u][:], xg_d[2 * u:2 * u + 2].rearrange("j p f -> p j f"))
            for e in range(E_LOC):
                for u in range(U):
                    nc.sync.dma_start(
                        w13[e][u][:],
                        w13_d[e, 2 * u:2 * u + 2].rearrange("j p f -> p j f"))
            for e in range(E_LOC):
                for h in range(2):
                    k0 = h * (IT // 2)
                    nc.sync.dma_start(
                        w2[e][h][:],
                        w2_d[e, k0:k0 + IT // 2].rearrange("k p f -> p k f"))

            # per-k aT tiles so mm2's early k-ops don't wait on late copies
            aT = [[wtsp.tile([128, cap], dt.bfloat16, tag=f"aT_{e}_{k}",
                             name=f"aT_{e}_{k}") for k in range(IT)]
                  for e in range(E_LOC)]
            yes = [[youtp.tile([128, D], dt.bfloat16, tag=f"ye_{e}_{m}", name=f"ye_{e}_{m}")
                    for m in range(MT)] for e in range(E_LOC)]

            # PE warm-up: keep the tensor engine continuously busy through
            # the initial DMA wait so the DVFS ramp completes before the
            # first real matmul.
            wps = pss.tile([128, 128], dt.bfloat16, tag="sm", name="warm_t")
            for _ in range(24):
                nc.tensor.transpose(wps[:], ident[:], ident[:])

            def mm1_and_silu(e):
                # h[tok, f] accumulated over d in DoubleRow ops; each
                # (m, fb) PSUM tile is one [128, gate256|up256] bank.
                # u is the outer loop so each w13 chunk is consumed at the
                # rate the DMA stream delivers it.
                hs = [[psh.tile([128, 512], dt.float32, tag="h",
                                name=f"h_{e}_{m}_{fb}") for fb in range(FB)]
                      for m in range(MT)]
                for u in range(U):
                    for m in range(MT):
                        stat = xg[u][:, :, e * cap + m * 128:e * cap + (m + 1) * 128]
                        for fb in range(FB):
                            nc.tensor.matmul(
                                hs[m][fb][:],
                                stat,
                                w13[e][u][:, :, fb * 512:(fb + 1) * 512],
                                start=(u == 0), stop=(u == U - 1),
                                perf_mode=mybir.MatmulPerfMode.DoubleRow)
                out = []
                for m in range(MT):
                    for fb in range(FB):
                        h = hs[m][fb]
                        s = sap.tile([128, 256], dt.float32, tag="silu")
                        nc.scalar.activation(
                            s[:], h[:, 0:256],
                            mybir.ActivationFunctionType.Silu)
                        a = ap_.tile([128, 256], dt.bfloat16, tag="a",
                                     name=f"a_{e}_{m}_{fb}")
                        # a = (silu(gate) * comb) * up
                        nc.vector.scalar_tensor_tensor(
                            a[:], s[:], combg[e][:, m, :], h[:, 256:512],
                            op0=mybir.AluOpType.mult,
                            op1=mybir.AluOpType.mult)
                        out.append((m, fb, a))
                return out

            def transposes(e, alist):
                # k-major so aT[e][k] tiles complete in mm2's consumption
                # order; copies alternate vector/scalar to halve the drain.
                byk = sorted(alist, key=lambda t: t[1])
                n = 0
                for m, fb, a in byk:
                    for half in range(2):
                        k = 2 * fb + half
                        pt = pss.tile([128, 128], dt.bfloat16, tag="sm")
                        nc.tensor.transpose(
                            pt[:], a[:, half * 128:(half + 1) * 128], ident[:])
                        if n % 2 == 0:
                            nc.vector.tensor_copy(
                                aT[e][k][:, m * 128:(m + 1) * 128], pt[:])
                        else:
                            nc.scalar.copy(
                                aT[e][k][:, m * 128:(m + 1) * 128], pt[:])
                        n += 1

            def mm2(e):
                for m in range(MT):
                    for dq in range(4):
                        yh = pss.tile([128, 512], dt.float32, tag="sm")
                        for k in range(IT):
                            nc.tensor.matmul(
                                yh[:],
                                aT[e][k][:, m * 128:(m + 1) * 128],
                                w2[e][k // 3][:, k % 3, dq * 512:(dq + 1) * 512],
                                start=(k == 0), stop=(k == IT - 1))
                        nc.scalar.copy(
                            yes[e][m][:, dq * 512:(dq + 1) * 512], yh[:])
                    nc.scalar.dma_start(
                        ye_d[e].rearrange("(m p) f -> p m f", p=128)[:, m, :],
                        yes[e][m][:])

            # PE order: mm1(e0), mm1(e1), transp(e0), mm2(e0), transp(e1), mm2(e1)
            a0 = mm1_and_silu(0)
            a1 = mm1_and_silu(1)
            transposes(0, a0)
            mm2(0)
            transposes(1, a1)
            mm2(1)

    nc.finalize()
    if split_waits:
        _split_excess_waits(nc)
    return nc


def kernel(hidden_states, topk_weights, topk_ids, w13_weight, w13_weight_scale,
           w2_weight, w2_weight_scale):
    from concourse.bass_utils import run_bass_kernel_spmd

    x = np.ascontiguousarray(hidden_states, dtype=np.float32)
    tw = np.asarray(topk_weights, dtype=np.float32)
    ti = np.asarray(topk_ids)

    # host routing: combine weights + per-expert token lists
    comb = np.zeros((T, E), np.float32)
    for k in range(TOPK):
        np.add.at(comb, (np.arange(T), ti[:, k]), tw[:, k])
    routed = comb > 0.0
    idx = [np.nonzero(routed[:, e])[0] for e in range(E)]
    counts = [len(ix) for ix in idx]
    cap = max(128, -(-max(counts) // 128) * 128)

    if cap not in _PROGRAM_CACHE:
        _PROGRAM_CACHE[cap] = _build_program(cap)
    nc = _PROGRAM_CACHE[cap]

    # staging quantization (exact reference replica) + lossless weight dequant
    x8T = _quant_dequant_fp8(x).astype(FP8).T  # [D, T]
    w13 = _dequant_mxfp4(np.asarray(w13_weight), np.asarray(w13_weight_scale))
    w2 = _dequant_mxfp4(np.asarray(w2_weight), np.asarray(w2_weight_scale))
    MT = cap // 128

    in_maps = []
    for core in range(N_CORES):
        xgT = np.zeros((D, E_LOC * cap), FP8)
        cg = np.zeros((E_LOC, cap), np.float32)
        w13t = np.zeros((E_LOC, DT, 128, 2 * I), FP8)
        w2t = np.zeros((E_LOC, IT, 128, D), FP8)
        for le in range(E_LOC):
            e = core * E_LOC + le
            ix = idx[e]
            xgT[:, le * cap:le * cap + len(ix)] = x8T[:, ix]
            cg[le, :len(ix)] = comb[ix, e]
            w13t[le] = w13[e][_FPERM].T.astype(FP8).reshape(DT, 128, 2 * I)
            w2t[le] = w2[e].T.astype(FP8).reshape(IT, 128, D)
        in_maps.append({
            "xg": np.ascontiguousarray(xgT.reshape(DT, 128, E_LOC * cap)),
            "comb": np.ascontiguousarray(cg.reshape(E_LOC, MT, 128, 1)),
            "w13t": w13t,
            "w2t": w2t,
        })

    res = run_bass_kernel_spmd(nc, in_maps, list(range(N_CORES)))

    out = np.zeros((T, D), np.float32)
    for core in range(N_CORES):
        ye = np.asarray(res.results[core]["ye"], dtype=np.float32)
        for le in range(E_LOC):
            e = core * E_LOC + le
            ix = idx[e]
            out[ix] += ye[le, :len(ix)]
    return out
